# revision 25
# baseline (speedup 1.0000x reference)
"""MoE routing kernel for Trainium2 (8 NeuronCores, expert-parallel).

out[i] = x[i] + relu(x[i] @ W[e].T + b[e]),  e = cam_pred_ids[i]

Strategy: route tokens by expert on the host, so core e computes ONLY
expert e's tokens with ONLY W[e] (8MB f16 / 4MB fp8 instead of 128MB).
The device computes hT[o, n] = relu(sum_k WT[k, o] * xT[k, n] + b[o])
with K on partitions for both operands; the residual (+x, exact f32)
is applied on the host during the unshard/scatter step.

Modes (BASS_MOE_MODE): "f16" (rel err ~1.5e-4) or "fp8" (e4m3 with
DoubleRow K=256 packing -> 2x PE throughput, rel err ~1.4e-2).

Loop structure: work is [column chunk <=512] x [ot group].  The FIRST
group spans ot 0-7 x chunk 0 on all 8 PSUM banks with the kt loop
outermost: its first matmul needs only the first k-tiles of x and of
the weight stream, and its steady HBM demand (~225 GB/s f16) stays
inside the per-core budget -- the PE starts ~1.5us after the DMA
triggers and doesn't starve.  Remaining groups are 4 ot x 1 chunk
(ping-ponging 4-bank PSUM sets) ordered so a fresh weight stream and
a fresh x chunk are never demanded in the same window.  The whole
weight stack stays resident in SBUF; x is loaded once.  Output (the
relu term only) is stored f16 and widened on the host.

Ring split (HWDGE rings exist only on SP/sync and Activation/scalar):
sync = weight stream (half 0 in pieces to race the PE) + output
stores; scalar = x (chunk 0 in pieces) + bias.
"""

import os
import numpy as np

import concourse.bass as bass
from concourse import bacc
import concourse.mybir as mybir
import concourse.tile as tile
from concourse.bass_utils import run_bass_kernel_spmd

MODE = os.environ.get("BASS_MOE_MODE", "fp8")

NUM_EXPERTS = 8
DIM = 2048
KT = DIM // 128   # 16 k-tiles
OT = DIM // 128   # 16 o-tiles
HB = 8            # o-tiles per half (first super-group spans one half)
OB = 4            # o-tiles per regular group (4 PSUM banks)

W_SCALE = 2.0 ** 13   # fp8: host pre-scale for W
X_SCALE = 2.0 ** 4    # fp8: host pre-scale for x


def _chunks(np_tokens: int) -> list[tuple[int, int]]:
    """Split the free dim into matmul chunks of <=512 (one PSUM bank),
    each >=232 when possible (below ~230 cols the 97ns LDWEIGHTS shadow
    outruns the matmul and the PE stalls on weight loads)."""
    out = []
    pos = 0
    rem = np_tokens
    while rem > 0:
        if rem > 512 + 232:
            take = 512
        elif rem > 512:
            take = (rem + 1) // 2  # two chunks, both >=232
        else:
            take = rem
        out.append((pos, take))
        pos += take
        rem -= take
    return out


def _build_nc(np_tokens: int, mode: str):
    f32 = mybir.dt.float32
    f16 = mybir.dt.float16
    fp8 = mode == "fp8"
    mm_dt = mybir.dt.float8e4 if fp8 else f16
    # fp8 packs 2 k-tiles per matmul (DoubleRow): the "kt" axis of the
    # weight stack halves and each entry carries a [2, 128] k-pair.
    KTW = KT // 2 if fp8 else KT  # weight-stack kt entries
    KP = 2 if fp8 else 1          # k-tiles per entry

    nc = bacc.Bacc()
    # wt[h, k, ktw, j, kp, o] = W[(h*8+j)*128+o, ((ktw*KP)+kp)*128 + k]
    # (pre-scaled by W_SCALE in fp8 mode): per partition k, one half is a
    # contiguous 16KB (f16) / 8KB (fp8) run in consumption order.  kp stays
    # a separate axis: DoubleRow lhsT APs must be [2, 128], not [256].
    wt_d = nc.declare_dram_parameter("wt", [2, 128, KTW, HB, KP, 128], mm_dt,
                                     isOutput=False)
    xt_d = nc.declare_dram_parameter("xt", [DIM, np_tokens], mm_dt,
                                     isOutput=False)
    b_d = nc.declare_dram_parameter("b", [128, OT], f32, isOutput=False)
    out_d = nc.declare_dram_parameter("out", [DIM, np_tokens], f16,
                                      isOutput=True)

    chunks = _chunks(np_tokens)
    relu = mybir.ActivationFunctionType.Relu
    act_scale = 1.0 / (W_SCALE * X_SCALE) if fp8 else 1.0
    perf_mode = mybir.MatmulPerfMode.DoubleRow if fp8 else None

    xt_r = xt_d.rearrange("(t p) n -> p t n", p=128)
    out_r = out_d.rearrange("(t p) n -> p t n", p=128)

    with tile.TileContext(nc) as tc:
        with (
            tc.tile_pool(name="wp", bufs=1) as wp,
            tc.tile_pool(name="xp", bufs=1) as xp,
            tc.tile_pool(name="bp", bufs=1) as bp,
            tc.tile_pool(name="op", bufs=3) as op,
            tc.tile_pool(name="pp", bufs=1, space="PSUM") as pp,
        ):
            wall = wp.tile([128, 2, KTW, HB, KP, 128], mm_dt, name="wall")
            xm = xp.tile([128, KT, np_tokens], mm_dt, name="xm")
            btile = bp.tile([128, OT], f32, name="btile")

            # Half 0 of the weights in pieces (smallest first, so the
            # super-group's first matmul waits ~0.8us, not for a 512KB
            # transfer); half 1 in two transfers (first consumed past the
            # midpoint, lands long before).
            w_pieces = [1, 1, 2, 2] + [2] * ((KTW - 6) // 2) if KTW >= 8 else [1] * KTW
            s = 0
            for pw in w_pieces:
                nc.sync.dma_start(out=wall[:, 0, s:s + pw],
                                  in_=wt_d[0, :, s:s + pw])
                s += pw
            nc.sync.dma_start(out=wall[:, 1, :KTW // 2],
                              in_=wt_d[1, :, :KTW // 2])
            nc.sync.dma_start(out=wall[:, 1, KTW // 2:],
                              in_=wt_d[1, :, KTW // 2:])

            # x chunk 0 in pieces, smallest first (consumed one k-tile per
            # super-group kt-step); later chunks whole.
            (c0, w0) = chunks[0]
            x_pieces = [2, 2, 4, 4, 4] if KT == 16 else [4] * (KT // 4)
            s = 0
            for pw in x_pieces:
                nc.scalar.dma_start(out=xm[:, s:s + pw, c0:c0 + w0],
                                    in_=xt_r[:, s:s + pw, c0:c0 + w0])
                s += pw
            nc.scalar.dma_start(out=btile, in_=b_d[:, :])
            for (cn, wn) in chunks[1:]:
                nc.scalar.dma_start(out=xm[:, :, cn:cn + wn],
                                    in_=xt_r[:, :, cn:cn + wn])

            def drain(ps_list, ots, cn, wn, store_each=False):
                otile = op.tile([128, len(ots), 512], f16, name="otile", tag="o")
                for oi, ot in enumerate(ots):
                    nc.scalar.activation(
                        otile[:, oi, :wn],
                        ps_list[oi][:, :wn],
                        relu,
                        bias=btile[:, ot:ot + 1],
                        scale=act_scale,
                    )
                    if store_each:
                        nc.sync.dma_start(
                            out=out_r[:, ot:ot + 1, cn:cn + wn],
                            in_=otile[:, oi:oi + 1, :wn],
                        )
                if not store_each:
                    nc.sync.dma_start(
                        out=out_r[:, ots[0]:ots[0] + len(ots), cn:cn + wn],
                        in_=otile[:, :, :wn],
                    )

            def mm_group(h, js, cn, wn, ps_list, ot_outer=False,
                         stagger_tail=False):
                # ot_outer staggers bank completion so the drain pipeline
                # overlaps the group's own matmuls (used for the last group,
                # where there is no following group to hide the drain under).
                # stagger_tail (super-group): banks 0-3 run their last two
                # kt-steps before banks 4-7, so their activations are done by
                # the time the next group reuses those PSUM banks.
                nb = len(js)
                if ot_outer:
                    order = [(ktw, oi) for oi in range(nb) for ktw in range(KTW)]
                elif stagger_tail and nb == 8:
                    order = [(ktw, oi) for ktw in range(KTW - 2)
                             for oi in range(nb)]
                    order += [(ktw, oi) for oi in range(4)
                              for ktw in (KTW - 2, KTW - 1)]
                    order += [(ktw, oi) for oi in range(4, 8)
                              for ktw in (KTW - 2, KTW - 1)]
                else:
                    order = [(ktw, oi) for ktw in range(KTW) for oi in range(nb)]
                for ktw, oi in order:
                    j = js[oi]
                    if fp8:
                        lhsT = wall[:, h, ktw, j]          # [128, 2, 128]
                        rhs = xm[:, KP * ktw:KP * (ktw + 1), cn:cn + wn]
                    else:
                        lhsT = wall[:, h, ktw, j, 0]       # [128, 128]
                        rhs = xm[:, ktw, cn:cn + wn]
                    nc.tensor.matmul(
                        ps_list[oi][:, :wn],
                        lhsT,
                        rhs,
                        start=(ktw == 0),
                        stop=(ktw == KTW - 1),
                        perf_mode=perf_mode,
                    )

            # PE warmup: dummy matmuls on zeroed SBUF while the first real
            # DMA pieces are still in flight (~3us queue cold-start), so the
            # DVFS clock is ramped before the first real matmul.
            dum = bp.tile([128, KP, 128], mm_dt, name="dum")
            nc.vector.memset(dum, 0)
            dps = pp.tile([128, 512], f32, name="dps", tag="ps0")
            for _ in range(30):
                nc.tensor.matmul(
                    dps[:, :128],
                    dum if fp8 else dum[:, 0],
                    dum if fp8 else dum[:, 0],
                    start=True, stop=True,
                    perf_mode=perf_mode,
                )

            # Super-group: ot 0-7 x chunk 0 on all 8 banks.
            ps8 = [pp.tile([128, 512], f32, name="ps", tag=f"ps{oi}")
                   for oi in range(HB)]
            mm_group(0, range(HB), c0, w0, ps8, stagger_tail=True)
            drain(ps8[:OB], list(range(0, OB)), c0, w0)
            drain(ps8[OB:], list(range(OB, HB)), c0, w0)

            # Remaining groups: 4 ot x 1 chunk; for each half, walk the
            # leftover (ot-block, chunk) pairs so fresh weights (next half)
            # and fresh x (later chunks) are never demanded together.
            rest = []
            for bj in (0, 1):          # ot blocks within half 0
                for ci, (cn, wn) in enumerate(chunks):
                    if ci == 0:
                        continue       # covered by the super-group
                    rest.append((0, bj, cn, wn))
            for bj in (0, 1):          # half 1: all chunks
                for (cn, wn) in chunks:
                    rest.append((1, bj, cn, wn))

            # 8 physical banks = 8 bufs=1 tags; regular groups ping-pong
            # between tag sets 0-3 and 4-7 (allocating a tag again waits for
            # its previous group's activation, i.e. the bank is drained).
            for gi, (h, bj, cn, wn) in enumerate(rest):
                js = range(bj * OB, (bj + 1) * OB)
                t0 = (gi % 2) * OB
                last = gi == len(rest) - 1
                ps4 = [pp.tile([128, 512], f32, name="ps", tag=f"ps{t0 + oi}")
                       for oi in range(OB)]
                # Last group: stagger bank completion (ot-outer) and store
                # per-ot so its drain pipelines with its own matmuls instead
                # of fully serializing after them.
                mm_group(h, js, cn, wn, ps4, ot_outer=last)
                drain(ps4, [h * HB + j for j in js], cn, wn, store_each=last)
    nc.compile()
    return nc


def kernel(x, cam_pred_ids, W, b, _want_results=False):
    x = np.ascontiguousarray(np.asarray(x), dtype=np.float32)
    W = np.asarray(W, dtype=np.float32)
    b = np.asarray(b, dtype=np.float32)
    ids = np.asarray(cam_pred_ids).astype(np.int64)

    counts = np.bincount(ids, minlength=NUM_EXPERTS)
    order = np.argsort(ids, kind="stable")
    np_tokens = max(512, int(counts.max()))

    mode = MODE
    fp8 = mode == "fp8"
    if fp8:
        import ml_dtypes
        # mybir.dt.np(float8e4) is the e4m3 (non-fn) variant; max normal 240,
        # our scaled values stay <=181.
        mm_np = ml_dtypes.float8_e4m3
        w_mult, x_mult = W_SCALE, X_SCALE
    else:
        mm_np = np.float16
        w_mult, x_mult = 1.0, 1.0
    KTW = KT // 2 if fp8 else KT
    KP = 2 if fp8 else 1

    # per-expert padded token index lists (pad with token 0; discarded later)
    starts = np.zeros(NUM_EXPERTS + 1, dtype=np.int64)
    np.cumsum(counts, out=starts[1:])
    idx = np.zeros((NUM_EXPERTS, np_tokens), dtype=np.int64)
    for e in range(NUM_EXPERTS):
        idx[e, : counts[e]] = order[starts[e] : starts[e + 1]]

    in_maps = []
    for e in range(NUM_EXPERTS):
        xg = x[idx[e]]  # [Np, DIM]
        xt = np.ascontiguousarray((xg.T * x_mult), dtype=mm_np)  # [DIM, Np]
        # wt[h, k, ktw, j, kp*128+o] = Ws[(h*8+j)*128+o, (ktw*KP+kp)*128+k]
        ws = W[e] * w_mult if fp8 else W[e]
        wdev = np.ascontiguousarray(
            ws.reshape(2, HB, 128, KTW, KP, 128).transpose(0, 5, 3, 1, 4, 2),
            dtype=mm_np,
        )
        in_maps.append({
            "wt": wdev,
            "xt": xt,
            "b": np.ascontiguousarray(b[e].reshape(OT, 128).T),
        })

    nc = _build_nc(np_tokens, mode)
    res = run_bass_kernel_spmd(
        nc,
        in_maps,
        core_ids=list(range(NUM_EXPERTS)),
        trace=bool(int(os.environ.get("BASS_MOE_TRACE", "0"))),
    )

    out = np.empty_like(x)
    for e in range(NUM_EXPERTS):
        he = res.results[e]["out"]  # [DIM, Np] f16: relu(x@W.T + b)
        valid = idx[e, : counts[e]]
        # residual applied host-side in f32
        out[valid] = x[valid] + he.T[: counts[e]].astype(np.float32)
    if _want_results:
        return out, res
    return out


# revision 26
# speedup vs baseline: 1.0552x; 1.0552x over previous
"""MoE routing kernel for Trainium2 (8 NeuronCores, expert-parallel).

out[i] = x[i] + relu(x[i] @ W[e].T + b[e]),  e = cam_pred_ids[i]

Strategy: route tokens by expert on the host, so core e computes ONLY
expert e's tokens with ONLY W[e] (8MB f16 / 4MB fp8 instead of 128MB).
The device computes hT[o, n] = relu(sum_k WT[k, o] * xT[k, n] + b[o])
with K on partitions for both operands; the residual (+x, exact f32)
is applied on the host during the unshard/scatter step.

Modes (BASS_MOE_MODE): "f16" (rel err ~1.5e-4) or "fp8" (e4m3 with
DoubleRow K=256 packing -> 2x PE throughput, rel err ~1.4e-2).

Loop structure: work is [column chunk <=512] x [ot group].  The FIRST
group spans ot 0-7 x chunk 0 on all 8 PSUM banks with the kt loop
outermost: its first matmul needs only the first k-tiles of x and of
the weight stream, and its steady HBM demand (~225 GB/s f16) stays
inside the per-core budget -- the PE starts ~1.5us after the DMA
triggers and doesn't starve.  Remaining groups are 4 ot x 1 chunk
(ping-ponging 4-bank PSUM sets) ordered so a fresh weight stream and
a fresh x chunk are never demanded in the same window.  The whole
weight stack stays resident in SBUF; x is loaded once.  Output (the
relu term only) is stored f16 and widened on the host.

Ring split (HWDGE rings exist only on SP/sync and Activation/scalar):
sync = weight stream (half 0 in pieces to race the PE) + output
stores; scalar = x (chunk 0 in pieces) + bias.
"""

import os
import numpy as np

import concourse.bass as bass
from concourse import bacc
import concourse.mybir as mybir
import concourse.tile as tile
from concourse.bass_utils import run_bass_kernel_spmd

MODE = os.environ.get("BASS_MOE_MODE", "fp8")

NUM_EXPERTS = 8
DIM = 2048
KT = DIM // 128   # 16 k-tiles
OT = DIM // 128   # 16 o-tiles
HB = 8            # o-tiles per half (first super-group spans one half)
OB = 4            # o-tiles per regular group (4 PSUM banks)

W_SCALE = 2.0 ** 13   # fp8: host pre-scale for W
X_SCALE = 2.0 ** 4    # fp8: host pre-scale for x


def _chunks(np_tokens: int) -> list[tuple[int, int]]:
    """Split the free dim into matmul chunks of <=512 (one PSUM bank),
    each >=232 when possible (below ~230 cols the 97ns LDWEIGHTS shadow
    outruns the matmul and the PE stalls on weight loads)."""
    out = []
    pos = 0
    rem = np_tokens
    while rem > 0:
        if rem > 512 + 232:
            take = 512
        elif rem > 512:
            take = (rem + 1) // 2  # two chunks, both >=232
        else:
            take = rem
        out.append((pos, take))
        pos += take
        rem -= take
    return out


def _build_nc(np_tokens: int, mode: str):
    f32 = mybir.dt.float32
    f16 = mybir.dt.float16
    fp8 = mode == "fp8"
    mm_dt = mybir.dt.float8e4 if fp8 else f16
    # fp8 packs 2 k-tiles per matmul (DoubleRow): the "kt" axis of the
    # weight stack halves and each entry carries a [2, 128] k-pair.
    KTW = KT // 2 if fp8 else KT  # weight-stack kt entries
    KP = 2 if fp8 else 1          # k-tiles per entry

    nc = bacc.Bacc()
    # wt[h, k, ktw, j, kp, o] = W[(h*8+j)*128+o, ((ktw*KP)+kp)*128 + k]
    # (pre-scaled by W_SCALE in fp8 mode): per partition k, one half is a
    # contiguous 16KB (f16) / 8KB (fp8) run in consumption order.  kp stays
    # a separate axis: DoubleRow lhsT APs must be [2, 128], not [256].
    wt_d = nc.declare_dram_parameter("wt", [2, 128, KTW, HB, KP, 128], mm_dt,
                                     isOutput=False)
    xt_d = nc.declare_dram_parameter("xt", [DIM, np_tokens], mm_dt,
                                     isOutput=False)
    b_d = nc.declare_dram_parameter("b", [128, OT], f32, isOutput=False)
    out_d = nc.declare_dram_parameter("out", [DIM, np_tokens], f16,
                                      isOutput=True)

    chunks = _chunks(np_tokens)
    relu = mybir.ActivationFunctionType.Relu
    act_scale = 1.0 / (W_SCALE * X_SCALE) if fp8 else 1.0
    perf_mode = mybir.MatmulPerfMode.DoubleRow if fp8 else None

    xt_r = xt_d.rearrange("(t p) n -> p t n", p=128)
    out_r = out_d.rearrange("(t p) n -> p t n", p=128)

    with tile.TileContext(nc) as tc:
        with (
            tc.tile_pool(name="wp", bufs=1) as wp,
            tc.tile_pool(name="xp", bufs=1) as xp,
            tc.tile_pool(name="bp", bufs=1) as bp,
            tc.tile_pool(name="op", bufs=3) as op,
            tc.tile_pool(name="pp", bufs=1, space="PSUM") as pp,
        ):
            wall = wp.tile([128, 2, KTW, HB, KP, 128], mm_dt, name="wall")
            xm = xp.tile([128, KT, np_tokens], mm_dt, name="xm")
            btile = bp.tile([128, OT], f32, name="btile")

            # Half 0 of the weights in pieces (smallest first, so the
            # super-group's first matmul waits ~0.8us, not for a 512KB
            # transfer); half 1 in two transfers (first consumed past the
            # midpoint, lands long before).
            w_pieces = [1, 1, 2, 2] + [2] * ((KTW - 6) // 2) if KTW >= 8 else [1] * KTW
            s = 0
            for pw in w_pieces:
                nc.sync.dma_start(out=wall[:, 0, s:s + pw],
                                  in_=wt_d[0, :, s:s + pw])
                s += pw
            nc.sync.dma_start(out=wall[:, 1, :KTW // 2],
                              in_=wt_d[1, :, :KTW // 2])
            nc.sync.dma_start(out=wall[:, 1, KTW // 2:],
                              in_=wt_d[1, :, KTW // 2:])

            # x chunk 0 in pieces, smallest first (consumed one k-tile per
            # super-group kt-step); later chunks whole.
            (c0, w0) = chunks[0]
            x_pieces = [2, 2, 4, 4, 4] if KT == 16 else [4] * (KT // 4)
            s = 0
            for pw in x_pieces:
                nc.scalar.dma_start(out=xm[:, s:s + pw, c0:c0 + w0],
                                    in_=xt_r[:, s:s + pw, c0:c0 + w0])
                s += pw
            nc.scalar.dma_start(out=btile, in_=b_d[:, :])
            for (cn, wn) in chunks[1:]:
                nc.scalar.dma_start(out=xm[:, :, cn:cn + wn],
                                    in_=xt_r[:, :, cn:cn + wn])

            def drain(ps_list, ots, cn, wn, store_each=False):
                otile = op.tile([128, len(ots), 512], f16, name="otile", tag="o")
                for oi, ot in enumerate(ots):
                    nc.scalar.activation(
                        otile[:, oi, :wn],
                        ps_list[oi][:, :wn],
                        relu,
                        bias=btile[:, ot:ot + 1],
                        scale=act_scale,
                    )
                    if store_each:
                        nc.sync.dma_start(
                            out=out_r[:, ot:ot + 1, cn:cn + wn],
                            in_=otile[:, oi:oi + 1, :wn],
                        )
                if not store_each:
                    nc.sync.dma_start(
                        out=out_r[:, ots[0]:ots[0] + len(ots), cn:cn + wn],
                        in_=otile[:, :, :wn],
                    )

            def mm_group(h, js, cn, wn, ps_list, ot_outer=False,
                         stagger_tail=False):
                # ot_outer staggers bank completion so the drain pipeline
                # overlaps the group's own matmuls (used for the last group,
                # where there is no following group to hide the drain under).
                # stagger_tail (super-group): banks 0-3 run their last two
                # kt-steps before banks 4-7, so their activations are done by
                # the time the next group reuses those PSUM banks.
                nb = len(js)
                if ot_outer:
                    order = [(ktw, oi) for oi in range(nb) for ktw in range(KTW)]
                elif stagger_tail and nb == 8:
                    order = [(ktw, oi) for ktw in range(KTW - 2)
                             for oi in range(nb)]
                    order += [(ktw, oi) for oi in range(4)
                              for ktw in (KTW - 2, KTW - 1)]
                    order += [(ktw, oi) for oi in range(4, 8)
                              for ktw in (KTW - 2, KTW - 1)]
                else:
                    order = [(ktw, oi) for ktw in range(KTW) for oi in range(nb)]
                for ktw, oi in order:
                    j = js[oi]
                    if fp8:
                        lhsT = wall[:, h, ktw, j]          # [128, 2, 128]
                        rhs = xm[:, KP * ktw:KP * (ktw + 1), cn:cn + wn]
                    else:
                        lhsT = wall[:, h, ktw, j, 0]       # [128, 128]
                        rhs = xm[:, ktw, cn:cn + wn]
                    nc.tensor.matmul(
                        ps_list[oi][:, :wn],
                        lhsT,
                        rhs,
                        start=(ktw == 0),
                        stop=(ktw == KTW - 1),
                        perf_mode=perf_mode,
                    )

            # PE warmup: dummy matmuls on zeroed SBUF while the first real
            # DMA pieces are still in flight (~3us queue cold-start), so the
            # DVFS clock is ramped before the first real matmul.
            dum = bp.tile([128, KP, 128], mm_dt, name="dum")
            nc.vector.memset(dum, 0)
            dps = pp.tile([128, 512], f32, name="dps", tag="ps0")
            for _ in range(30):
                nc.tensor.matmul(
                    dps[:, :128],
                    dum if fp8 else dum[:, 0],
                    dum if fp8 else dum[:, 0],
                    start=True, stop=True,
                    perf_mode=perf_mode,
                )

            # Super-group: ot 0-7 x chunk 0 on all 8 banks.
            ps8 = [pp.tile([128, 512], f32, name="ps", tag=f"ps{oi}")
                   for oi in range(HB)]
            mm_group(0, range(HB), c0, w0, ps8, stagger_tail=True)
            drain(ps8[:OB], list(range(0, OB)), c0, w0)
            drain(ps8[OB:], list(range(OB, HB)), c0, w0)

            # Remaining groups: 4 ot x 1 chunk.  The two half-1 chunk-0
            # groups (512 cols, 6.9us each) go FIRST: the super-group's 8
            # serial activations (~5.3us on scalar) outlast a narrow-chunk
            # group, so a 264-col group right after the super-group would
            # stall on undrained PSUM banks.  Narrow chunks follow once the
            # drain cadence has slack; fresh weights (half 1) and fresh x
            # (later chunks) are still never demanded in the same window.
            rest = []
            for bj in (0, 1):          # half 1, chunk 0 (512-col groups)
                rest.append((1, bj, c0, w0))
            for h in (0, 1):           # both halves: the narrow chunks
                for bj in (0, 1):
                    for ci, (cn, wn) in enumerate(chunks):
                        if ci == 0:
                            continue
                        rest.append((h, bj, cn, wn))

            # 8 physical banks = 8 bufs=1 tags; regular groups ping-pong
            # between tag sets 0-3 and 4-7 (allocating a tag again waits for
            # its previous group's activation, i.e. the bank is drained).
            for gi, (h, bj, cn, wn) in enumerate(rest):
                js = range(bj * OB, (bj + 1) * OB)
                t0 = (gi % 2) * OB
                last = gi == len(rest) - 1
                ps4 = [pp.tile([128, 512], f32, name="ps", tag=f"ps{t0 + oi}")
                       for oi in range(OB)]
                # Last group: stagger bank completion (ot-outer) and store
                # per-ot so its drain pipelines with its own matmuls instead
                # of fully serializing after them.
                mm_group(h, js, cn, wn, ps4, ot_outer=last)
                drain(ps4, [h * HB + j for j in js], cn, wn, store_each=last)
    nc.compile()
    return nc


def kernel(x, cam_pred_ids, W, b, _want_results=False):
    x = np.ascontiguousarray(np.asarray(x), dtype=np.float32)
    W = np.asarray(W, dtype=np.float32)
    b = np.asarray(b, dtype=np.float32)
    ids = np.asarray(cam_pred_ids).astype(np.int64)

    counts = np.bincount(ids, minlength=NUM_EXPERTS)
    order = np.argsort(ids, kind="stable")
    np_tokens = max(512, int(counts.max()))

    mode = MODE
    fp8 = mode == "fp8"
    if fp8:
        import ml_dtypes
        # mybir.dt.np(float8e4) is the e4m3 (non-fn) variant; max normal 240,
        # our scaled values stay <=181.
        mm_np = ml_dtypes.float8_e4m3
        w_mult, x_mult = W_SCALE, X_SCALE
    else:
        mm_np = np.float16
        w_mult, x_mult = 1.0, 1.0
    KTW = KT // 2 if fp8 else KT
    KP = 2 if fp8 else 1

    # per-expert padded token index lists (pad with token 0; discarded later)
    starts = np.zeros(NUM_EXPERTS + 1, dtype=np.int64)
    np.cumsum(counts, out=starts[1:])
    idx = np.zeros((NUM_EXPERTS, np_tokens), dtype=np.int64)
    for e in range(NUM_EXPERTS):
        idx[e, : counts[e]] = order[starts[e] : starts[e + 1]]

    in_maps = []
    for e in range(NUM_EXPERTS):
        xg = x[idx[e]]  # [Np, DIM]
        xt = np.ascontiguousarray((xg.T * x_mult), dtype=mm_np)  # [DIM, Np]
        # wt[h, k, ktw, j, kp*128+o] = Ws[(h*8+j)*128+o, (ktw*KP+kp)*128+k]
        ws = W[e] * w_mult if fp8 else W[e]
        wdev = np.ascontiguousarray(
            ws.reshape(2, HB, 128, KTW, KP, 128).transpose(0, 5, 3, 1, 4, 2),
            dtype=mm_np,
        )
        in_maps.append({
            "wt": wdev,
            "xt": xt,
            "b": np.ascontiguousarray(b[e].reshape(OT, 128).T),
        })

    nc = _build_nc(np_tokens, mode)
    res = run_bass_kernel_spmd(
        nc,
        in_maps,
        core_ids=list(range(NUM_EXPERTS)),
        trace=bool(int(os.environ.get("BASS_MOE_TRACE", "0"))),
    )

    out = np.empty_like(x)
    for e in range(NUM_EXPERTS):
        he = res.results[e]["out"]  # [DIM, Np] f16: relu(x@W.T + b)
        valid = idx[e, : counts[e]]
        # residual applied host-side in f32
        out[valid] = x[valid] + he.T[: counts[e]].astype(np.float32)
    if _want_results:
        return out, res
    return out
